# revision 6
# baseline (speedup 1.0000x reference)
"""Block-diagonal linear layer (8 x [256,256] blocks) on 8 Trainium2 cores.

out = block_diag(blocks) @ inp,  inp [2048, 16384] f32, blocks [8, 256, 256] f32.

Sharding: block-parallel — core c owns block c entirely: it loads only
blocks[c] (128 KiB fp16 vs 1 MiB replicated), input rows [c*256,(c+1)*256)
over the full 16384-column batch, and writes the matching output rows.
Per-core HBM traffic: 8.39 MiB in + 8.39 MiB out + 0.125 MiB weights.

Numerics: fp16 in/out with fp32 PSUM accumulation; end-to-end relative
L2 error ~3.6e-4 (gate is 2e-2).

Layout: host packs so every DMA is [128 partitions x 8 KiB contiguous]:
  x[p, (s*2+k)*2048 + b]  = inp[c*256 + k*128 + p, s*2048 + b]
  y[p, (s*2+mi)*2048 + b] = out[c*256 + mi*128 + p, s*2048 + b]
  w[p, (k*2+mi)*128 + m]  = blocks[c, mi*128 + m, k*128 + p]   (lhsT tiles)

Schedule notes (from trace analysis of the previous revisions):
- The PE pitch for a self-loading matmul is ~259 ns = 512-cycle matmul
  + ~128-cycle LDWEIGHTS, serialized on the array. Four consecutive
  matmuls share each weight tile, so after TileContext lowering we strip
  the redundant InstLdweights (keeping any that carry sync or new
  weights): pitch drops toward ~230 ns, saving ~4 us over 128 matmuls.
- Loads ride the SP HWDGE ring; stores ride the GpSimd SWDGE ring
  (Q7 descriptor-gen runs async, ~25 ns dispatch on the Pool sequencer —
  an ACT-ring store would block the Scalar sequencer, which we need for
  PSUM evictions). Final-stage stores are split 4 x 256 KiB across the
  ACT + SWDGE rings so the last bytes land right after the last eviction.
- PSUM: per mi-group two 2-bank tiles (bufs=4 = all 8 banks); mi0's
  evictions overlap mi1's matmuls so banks recycle without stalling the PE.
- Warmup: the PE clock ramps to full speed only after ~3.5 us of busy
  (HAM gate). Dummy matmuls gated only on a small DVE memset start that
  clock immediately at kernel entry, overlapping the first loads.
"""

import numpy as np

N_BLOCKS = 8
D = 256            # block dim
N = N_BLOCKS * D   # 2048
BATCH = 16384
NCORES = 8
P = 128
FREE = 512         # matmul moving free dim (= one fp32 PSUM bank)
CB = 2048          # batch columns per stage
NSTAGE = BATCH // CB  # 8
NJ = CB // FREE    # 4 matmul chunks per (stage, mi, k)

_CACHE = {}


def _strip_redundant_ldweights(nc, mybir):
    """Remove InstLdweights whose weights AP matches the previous PE weight
    load and that carry no sync — the PE weight register already holds the
    right values, and each skipped LDWEIGHTS saves ~128 PE cycles."""
    removed = 0
    for b in nc.m.functions[0].blocks:
        prev_sig = None
        to_remove = []
        for ins in b.instructions:
            if getattr(ins, "engine", None) != mybir.EngineType.PE:
                continue
            if isinstance(ins, mybir.InstLdweights):
                sig = str(ins.ins[0])
                si = ins.sync_info
                has_sync = si is not None and (
                    len(si.on_wait) > 0 or len(si.on_update) > 0)
                if sig == prev_sig and not has_sync:
                    to_remove.append(ins)
                prev_sig = sig
            elif not isinstance(ins, mybir.InstMatmult):
                prev_sig = None
        for ins in to_remove:
            b.instructions.remove(ins)
        removed += len(to_remove)
    return removed


def _build_blockpar(mm_dtype_name: str = "float16"):
    import concourse.bacc as bacc
    import concourse.mybir as mybir
    import concourse.tile as tile

    mm_dt = getattr(mybir.dt, mm_dtype_name)
    nc = bacc.Bacc()
    inp = nc.declare_dram_parameter("inp", [P, 2 * NSTAGE * CB], mm_dt, isOutput=False)
    wt = nc.declare_dram_parameter("wt", [P, 4 * P], mm_dt, isOutput=False)
    out = nc.declare_dram_parameter("out", [P, 2 * NSTAGE * CB], mm_dt, isOutput=True)

    with tile.TileContext(nc) as tc:
        with (
            tc.tile_pool(name="w", bufs=1) as wpool,
            tc.tile_pool(name="x", bufs=NSTAGE) as xpool,
            tc.tile_pool(name="y", bufs=NSTAGE) as ypool,
            tc.tile_pool(name="ps", bufs=4, space="PSUM") as pspool,
        ):
            # PE warmup: dummy matmuls gated only on a small DVE memset start
            # the HAM busy-clock right at kernel entry, before any DMA lands.
            warm_w = wpool.tile([P, P], mm_dt, tag="warmw")
            warm_x = wpool.tile([P, FREE], mm_dt, tag="warmx")
            nc.vector.memset(warm_w[:], 0.0)
            nc.vector.memset(warm_x[:], 0.0)
            warm_ps = pspool.tile([P, FREE], mybir.dt.float32, tag="ps",
                                  name="warm_ps")
            N_WARM = 10
            for i in range(N_WARM):
                nc.tensor.matmul(
                    warm_ps[:], warm_w[:], warm_x[:],
                    start=(i == 0), stop=(i == N_WARM - 1),
                )

            # Weights resident in SBUF: [128, 512] = 1 KiB/partition.
            w_all = wpool.tile([P, 4 * P], mm_dt)
            nc.sync.dma_start(out=w_all[:], in_=wt[:])

            # All loads issued upfront on TWO HWDGE rings (SP + ACT), even
            # stages on SP, odd on ACT. The SDMA engines round-robin between
            # queues with pending work, so two load queues keep a ~2/3 share
            # of the ~425 GB/s per-core fabric once the store queue opens —
            # one queue alone starves the PE in the late stages.
            xts = []
            for s in range(NSTAGE):
                xt = xpool.tile([P, 2 * CB], mm_dt, tag="x", name=f"x_{s}")
                xts.append(xt)
                lo, hi = (2 * s) * CB, (2 * s + 2) * CB
                if s == 0:
                    # Split the first load across both rings so the first
                    # matmuls can start a half-transfer earlier.
                    nc.sync.dma_start(out=xt[:, :CB], in_=inp[:, :CB])
                    nc.scalar.dma_start(out=xt[:, CB:], in_=inp[:, CB : 2 * CB])
                elif s % 2 == 0:
                    nc.sync.dma_start(out=xt[:], in_=inp[:, lo:hi])
                else:
                    nc.scalar.dma_start(out=xt[:], in_=inp[:, lo:hi])

            H = CB // 2  # 1024
            for s in range(NSTAGE):
                xt = xts[s]
                yt = ypool.tile([P, 2 * CB], mm_dt, tag="y")
                for mi in range(2):
                    pss = [pspool.tile([P, 2 * FREE], mybir.dt.float32, tag="ps",
                                       name=f"ps_{s}_{mi}_{h}")
                           for h in range(2)]
                    for k in range(2):
                        col = (k * 2 + mi) * P
                        for j in range(NJ):
                            nc.tensor.matmul(
                                pss[j // 2][:, (j % 2) * FREE : (j % 2 + 1) * FREE],
                                w_all[:, col : col + P],
                                xt[:, k * CB + j * FREE : k * CB + (j + 1) * FREE],
                                start=(k == 0),
                                stop=(k == 1),
                                skip_group_check=True,
                            )
                    # Evictions on Vector and Scalar in parallel (~1.4us each
                    # for [128,1024] fp32 -> fp16).
                    dst = yt[:, mi * CB : (mi + 1) * CB]
                    nc.vector.tensor_copy(dst[:, :H], pss[0][:])
                    nc.scalar.copy(dst[:, H:], pss[1][:])
                    if s == NSTAGE - 1:
                        # Final stores: 256 KiB per eviction. mi0 rides the
                        # (already warm) SWDGE ring; mi1 — the true tail —
                        # rides the ACT HWDGE ring for the fastest landing.
                        base = (2 * s) * CB + mi * CB
                        eng0 = nc.gpsimd if mi == 0 else nc.scalar
                        eng1 = nc.gpsimd if mi == 0 else nc.scalar
                        eng0.dma_start(out=out[:, base : base + H],
                                       in_=dst[:, :H])
                        eng1.dma_start(out=out[:, base + H : base + CB],
                                       in_=dst[:, H:])
                if s != NSTAGE - 1:
                    # Stores ride the GpSimd SWDGE ring: loads keep the SP
                    # ring, and the Scalar sequencer stays free for PSUM
                    # evictions.
                    nc.gpsimd.dma_start(
                        out=out[:, (2 * s) * CB : (2 * s + 2) * CB], in_=yt[:]
                    )

    _strip_redundant_ldweights(nc, mybir)
    nc.compile()
    return nc


def _get_nc(key: str):
    if key not in _CACHE:
        _CACHE[key] = _build_blockpar(key)
    return _CACHE[key]


LAST_RESULTS = None  # BassKernelResults of the most recent run (for test.py)


def kernel(inp: np.ndarray, blocks: np.ndarray, _trace: bool = False,
           _mm_dtype: str = "float16") -> np.ndarray:
    global LAST_RESULTS
    import concourse.mybir as mybir
    from concourse.bass_utils import run_bass_kernel_spmd

    nc = _get_nc(_mm_dtype)
    np_dt = mybir.dt.np(getattr(mybir.dt, _mm_dtype))

    inp = np.asarray(inp, dtype=np.float32)
    blocks = np.asarray(blocks, dtype=np.float32)
    # x_packed[c][p, (s*2+k)*CB + b] = inp[c*256 + k*128 + p, s*CB + b]
    v = inp.reshape(NCORES, 2, P, NSTAGE, CB).astype(np_dt)   # [c, k, p, s, b]
    x_packed = np.ascontiguousarray(
        v.transpose(0, 2, 3, 1, 4).reshape(NCORES, P, 2 * NSTAGE * CB))
    # w_all[c][p, (k*2+mi)*128 + m] = blocks[c, mi*128+m, k*128+p]
    bt = blocks.transpose(0, 2, 1).astype(np_dt)              # [c, k*128+p, mi*128+m]
    w_all = np.ascontiguousarray(
        bt.reshape(NCORES, 2, P, 2, P).transpose(0, 2, 1, 3, 4).reshape(NCORES, P, 4 * P))

    in_maps = [{"inp": x_packed[c], "wt": w_all[c]} for c in range(NCORES)]
    res = None
    for attempt in range(3):
        try:
            res = run_bass_kernel_spmd(
                nc, in_maps, core_ids=list(range(NCORES)), trace=_trace
            )
            break
        except Exception:
            # Transient device wedges (NRT_EXEC_UNIT_UNRECOVERABLE) clear on
            # retry; re-raise only if persistent.
            if attempt == 2:
                raise
    LAST_RESULTS = res
    # unpack: y[c][p, (s*2+mi)*CB + b] -> out[c*256 + mi*128 + p, s*CB + b]
    y = np.stack([res.results[c]["out"] for c in range(NCORES)])
    y = y.reshape(NCORES, P, NSTAGE, 2, CB).astype(np.float32)  # [c, p, s, mi, b]
    out = y.transpose(0, 3, 1, 2, 4).reshape(N, BATCH)          # [c, mi, p, s, b]
    return np.ascontiguousarray(out)


# revision 7
# speedup vs baseline: 1.1069x; 1.1069x over previous
"""Block-diagonal linear layer (8 x [256,256] blocks) on 8 Trainium2 cores.

out = block_diag(blocks) @ inp,  inp [2048, 16384] f32, blocks [8, 256, 256] f32.

Sharding: block-parallel — core c owns block c entirely: it loads only
blocks[c] (128 KiB fp16 vs 1 MiB replicated), input rows [c*256,(c+1)*256)
over the full 16384-column batch, and writes the matching output rows.
Per-core HBM traffic: 8.39 MiB in + 8.39 MiB out + 0.125 MiB weights.

Numerics: fp16 in/out with fp32 PSUM accumulation; end-to-end relative
L2 error ~3.6e-4 (gate is 2e-2).

Layout: host packs so every DMA is [128 partitions x 8 KiB contiguous]:
  x[p, (s*2+k)*2048 + b]  = inp[c*256 + k*128 + p, s*2048 + b]
  y[p, (s*2+mi)*2048 + b] = out[c*256 + mi*128 + p, s*2048 + b]
  w[p, (k*2+mi)*128 + m]  = blocks[c, mi*128 + m, k*128 + p]   (lhsT tiles)

Schedule notes (from trace analysis of the previous revisions):
- The PE pitch for a self-loading matmul is ~259 ns = 512-cycle matmul
  + ~128-cycle LDWEIGHTS, serialized on the array. Four consecutive
  matmuls share each weight tile, so after TileContext lowering we strip
  the redundant InstLdweights (keeping any that carry sync or new
  weights): pitch drops toward ~230 ns, saving ~4 us over 128 matmuls.
- Loads ride the SP HWDGE ring; stores ride the GpSimd SWDGE ring
  (Q7 descriptor-gen runs async, ~25 ns dispatch on the Pool sequencer —
  an ACT-ring store would block the Scalar sequencer, which we need for
  PSUM evictions). Final-stage stores are split 4 x 256 KiB across the
  ACT + SWDGE rings so the last bytes land right after the last eviction.
- PSUM: per mi-group two 2-bank tiles (bufs=4 = all 8 banks); mi0's
  evictions overlap mi1's matmuls so banks recycle without stalling the PE.
- Warmup: the PE clock ramps to full speed only after ~3.5 us of busy
  (HAM gate). Dummy matmuls gated only on a small DVE memset start that
  clock immediately at kernel entry, overlapping the first loads.
"""

import numpy as np

N_BLOCKS = 8
D = 256            # block dim
N = N_BLOCKS * D   # 2048
BATCH = 16384
NCORES = 8
P = 128
FREE = 512         # matmul moving free dim (= one fp32 PSUM bank)
CB = 2048          # batch columns per stage
NSTAGE = BATCH // CB  # 8
NJ = CB // FREE    # 4 matmul chunks per (stage, mi, k)

_CACHE = {}


def _strip_redundant_ldweights(nc, mybir):
    """Remove InstLdweights whose weights AP matches the previous PE weight
    load and that carry no sync — the PE weight register already holds the
    right values, and each skipped LDWEIGHTS saves ~128 PE cycles."""
    removed = 0
    for b in nc.m.functions[0].blocks:
        prev_sig = None
        to_remove = []
        for ins in b.instructions:
            if getattr(ins, "engine", None) != mybir.EngineType.PE:
                continue
            if isinstance(ins, mybir.InstLdweights):
                sig = str(ins.ins[0])
                si = ins.sync_info
                has_sync = si is not None and (
                    len(si.on_wait) > 0 or len(si.on_update) > 0)
                if sig == prev_sig and not has_sync:
                    to_remove.append(ins)
                prev_sig = sig
            elif not isinstance(ins, mybir.InstMatmult):
                prev_sig = None
        for ins in to_remove:
            b.instructions.remove(ins)
        removed += len(to_remove)
    return removed


def _build_blockpar(mm_dtype_name: str = "float16"):
    import concourse.bacc as bacc
    import concourse.mybir as mybir
    import concourse.tile as tile

    mm_dt = getattr(mybir.dt, mm_dtype_name)
    nc = bacc.Bacc()
    inp = nc.declare_dram_parameter("inp", [P, 2 * NSTAGE * CB], mm_dt, isOutput=False)
    wt = nc.declare_dram_parameter("wt", [P, 4 * P], mm_dt, isOutput=False)
    out = nc.declare_dram_parameter("out", [P, 2 * NSTAGE * CB], mm_dt, isOutput=True)

    with tile.TileContext(nc) as tc:
        with (
            tc.tile_pool(name="w", bufs=1) as wpool,
            tc.tile_pool(name="x", bufs=NSTAGE) as xpool,
            tc.tile_pool(name="y", bufs=NSTAGE) as ypool,
            tc.tile_pool(name="ps", bufs=4, space="PSUM") as pspool,
        ):
            # PE warmup: dummy matmuls gated only on a small DVE memset start
            # the HAM busy-clock right at kernel entry, before any DMA lands.
            warm_w = wpool.tile([P, P], mm_dt, tag="warmw")
            warm_x = wpool.tile([P, FREE], mm_dt, tag="warmx")
            nc.vector.memset(warm_w[:], 0.0)
            nc.vector.memset(warm_x[:], 0.0)
            warm_ps = pspool.tile([P, FREE], mybir.dt.float32, tag="ps",
                                  name="warm_ps")
            N_WARM = 10
            for i in range(N_WARM):
                nc.tensor.matmul(
                    warm_ps[:], warm_w[:], warm_x[:],
                    start=(i == 0), stop=(i == N_WARM - 1),
                )

            # Weights resident in SBUF: [128, 512] = 1 KiB/partition.
            w_all = wpool.tile([P, 4 * P], mm_dt)
            nc.sync.dma_start(out=w_all[:], in_=wt[:])

            # All loads issued upfront on TWO HWDGE rings (SP + ACT), even
            # stages on SP, odd on ACT. The SDMA engines round-robin between
            # queues with pending work, so two load queues keep a ~2/3 share
            # of the ~425 GB/s per-core fabric once the store queue opens —
            # one queue alone starves the PE in the late stages.
            xts = []
            for s in range(NSTAGE):
                xt = xpool.tile([P, 2 * CB], mm_dt, tag="x", name=f"x_{s}")
                xts.append(xt)
                lo = (2 * s) * CB
                # Each stage's k0 half rides SP and its k1 half rides ACT:
                # both queues deliver the same stage concurrently, so stage
                # arrival keeps the full 2-queue share of the fabric.
                nc.sync.dma_start(out=xt[:, :CB], in_=inp[:, lo : lo + CB])
                nc.scalar.dma_start(out=xt[:, CB:], in_=inp[:, lo + CB : lo + 2 * CB])

            H = CB // 2  # 1024
            for s in range(NSTAGE):
                xt = xts[s]
                yt = ypool.tile([P, 2 * CB], mm_dt, tag="y")
                for mi in range(2):
                    pss = [pspool.tile([P, 2 * FREE], mybir.dt.float32, tag="ps",
                                       name=f"ps_{s}_{mi}_{h}")
                           for h in range(2)]
                    for k in range(2):
                        col = (k * 2 + mi) * P
                        for j in range(NJ):
                            nc.tensor.matmul(
                                pss[j // 2][:, (j % 2) * FREE : (j % 2 + 1) * FREE],
                                w_all[:, col : col + P],
                                xt[:, k * CB + j * FREE : k * CB + (j + 1) * FREE],
                                start=(k == 0),
                                stop=(k == 1),
                                skip_group_check=True,
                            )
                    # Evictions on Vector and Scalar in parallel (~1.4us each
                    # for [128,1024] fp32 -> fp16).
                    dst = yt[:, mi * CB : (mi + 1) * CB]
                    nc.vector.tensor_copy(dst[:, :H], pss[0][:])
                    nc.scalar.copy(dst[:, H:], pss[1][:])
                    if s == NSTAGE - 1:
                        # Final stores: 256 KiB per eviction. mi0 rides the
                        # (already warm) SWDGE ring; mi1 — the true tail —
                        # rides the ACT HWDGE ring for the fastest landing.
                        base = (2 * s) * CB + mi * CB
                        eng0 = nc.gpsimd if mi == 0 else nc.scalar
                        eng1 = nc.gpsimd if mi == 0 else nc.scalar
                        eng0.dma_start(out=out[:, base : base + H],
                                       in_=dst[:, :H])
                        eng1.dma_start(out=out[:, base + H : base + CB],
                                       in_=dst[:, H:])
                if s != NSTAGE - 1:
                    # Stores ride the GpSimd SWDGE ring: loads keep the SP
                    # ring, and the Scalar sequencer stays free for PSUM
                    # evictions.
                    nc.gpsimd.dma_start(
                        out=out[:, (2 * s) * CB : (2 * s + 2) * CB], in_=yt[:]
                    )

    _strip_redundant_ldweights(nc, mybir)
    nc.compile()
    return nc


def _get_nc(key: str):
    if key not in _CACHE:
        _CACHE[key] = _build_blockpar(key)
    return _CACHE[key]


LAST_RESULTS = None  # BassKernelResults of the most recent run (for test.py)


def kernel(inp: np.ndarray, blocks: np.ndarray, _trace: bool = False,
           _mm_dtype: str = "float16") -> np.ndarray:
    global LAST_RESULTS
    import concourse.mybir as mybir
    from concourse.bass_utils import run_bass_kernel_spmd

    nc = _get_nc(_mm_dtype)
    np_dt = mybir.dt.np(getattr(mybir.dt, _mm_dtype))

    inp = np.asarray(inp, dtype=np.float32)
    blocks = np.asarray(blocks, dtype=np.float32)
    # x_packed[c][p, (s*2+k)*CB + b] = inp[c*256 + k*128 + p, s*CB + b]
    v = inp.reshape(NCORES, 2, P, NSTAGE, CB).astype(np_dt)   # [c, k, p, s, b]
    x_packed = np.ascontiguousarray(
        v.transpose(0, 2, 3, 1, 4).reshape(NCORES, P, 2 * NSTAGE * CB))
    # w_all[c][p, (k*2+mi)*128 + m] = blocks[c, mi*128+m, k*128+p]
    bt = blocks.transpose(0, 2, 1).astype(np_dt)              # [c, k*128+p, mi*128+m]
    w_all = np.ascontiguousarray(
        bt.reshape(NCORES, 2, P, 2, P).transpose(0, 2, 1, 3, 4).reshape(NCORES, P, 4 * P))

    in_maps = [{"inp": x_packed[c], "wt": w_all[c]} for c in range(NCORES)]
    res = None
    for attempt in range(3):
        try:
            res = run_bass_kernel_spmd(
                nc, in_maps, core_ids=list(range(NCORES)), trace=_trace
            )
            break
        except Exception:
            # Transient device wedges (NRT_EXEC_UNIT_UNRECOVERABLE) clear on
            # retry; re-raise only if persistent.
            if attempt == 2:
                raise
    LAST_RESULTS = res
    # unpack: y[c][p, (s*2+mi)*CB + b] -> out[c*256 + mi*128 + p, s*CB + b]
    y = np.stack([res.results[c]["out"] for c in range(NCORES)])
    y = y.reshape(NCORES, P, NSTAGE, 2, CB).astype(np.float32)  # [c, p, s, mi, b]
    out = y.transpose(0, 3, 1, 2, 4).reshape(N, BATCH)          # [c, mi, p, s, b]
    return np.ascontiguousarray(out)
